# revision 11
# baseline (speedup 1.0000x reference)
"""GumbelSelector Trainium2 kernel.

Math: h = relu(s @ W1 + b1); lo = h @ W2 + b2  (2 classes)
  dec  = (argmax(lo) == 1)  ==  (z > 0)         where z = h @ (W2[:,1]-W2[:,0]) + (b2[1]-b2[0])
  prob = softmax(lo)[..., 1] ==  sigmoid(z)
  Per-row correction (LB=1): if a row of dec is all zero, activate argmax(rnoise).

Sharding: data-parallel over batch B=64 -> 8 cores x 8 rows. Weights replicated.

Device computes z for every token with fp16 operands (f32 PSUM accumulation);
fp16 keeps the matmul at 1 cycle/row (fp32 is 4) and halves HBM traffic. Host
pre-transposes each core's s shard to [D=256, 32768] fp16 so DMA loads are
coalesced with the contraction dim on SBUF partitions, then finishes
elementwise: prob = sigmoid(z), dec = z > 0, an exact f64 recompute of the
~0.5% of tokens with |z| < tau (fp16 max z error is ~1.6e-3, tau = 5e-3), and
the LB row correction.
"""

import sys

if "/opt/trn_rl_repo" not in sys.path:
    sys.path.insert(0, "/opt/trn_rl_repo")

import numpy as np

import concourse.bass as bass
import concourse.mybir as mybir
import concourse.tile as tile
from concourse import bacc
from concourse.bass_utils import run_bass_kernel_spmd

B, N, D = 64, 4096, 256
HID = D // 2  # 128
NCORES = 8
BPC = B // NCORES          # batch rows per core
TOK = BPC * N              # 32768 tokens per core
SLAB = 4096                # tokens per DMA slab (8 KiB/partition fp16 load)
TS = 512                   # tokens per compute tile (1 PSUM bank)
TAU = 5e-3                 # |z| window for exact host recompute
F32 = mybir.dt.float32
F16 = mybir.dt.float16

_NC = None


def _build_nc():
    nc = bacc.Bacc("TRN2", target_bir_lowering=False, debug=False)
    sT = nc.dram_tensor("sT", [D, TOK], F16, kind="ExternalInput")
    w1 = nc.dram_tensor("w1", [D, HID], F16, kind="ExternalInput")
    b1 = nc.dram_tensor("b1", [HID, 1], F32, kind="ExternalInput")
    w2 = nc.dram_tensor("w2", [HID, 1], F16, kind="ExternalInput")
    zout = nc.dram_tensor("zout", [1, TOK], F16, kind="ExternalOutput")

    AF = mybir.ActivationFunctionType

    with tile.TileContext(nc) as tc:
        with (
            tc.tile_pool(name="consts", bufs=1) as consts,
            tc.tile_pool(name="sload", bufs=4) as sload,
            tc.tile_pool(name="hpool", bufs=3) as hpool,
            tc.tile_pool(name="phpool", bufs=3, space=bass.MemorySpace.PSUM) as phpool,
            tc.tile_pool(name="pzpool", bufs=4, space=bass.MemorySpace.PSUM) as pzpool,
            tc.tile_pool(name="pwpool", bufs=1, space=bass.MemorySpace.PSUM) as pwpool,
        ):
            w1a = consts.tile([128, HID], F16)
            nc.gpsimd.dma_start(w1a[:], w1[0:128, :])
            w1b = consts.tile([128, HID], F16)
            nc.gpsimd.dma_start(w1b[:], w1[128:256, :])
            b1s = consts.tile([HID, 1], F32)
            nc.gpsimd.dma_start(b1s[:], b1[:])
            w2s = consts.tile([HID, 1], F16)
            nc.gpsimd.dma_start(w2s[:], w2[:])
            # z accumulates here (partition 0) and drains in big chunks so
            # tiny DMAs never pollute the two HWDGE FIFO rings. fp16 is
            # plenty: z's sign and |z| < tau detection need ~5e-4 near 0.
            zbig = consts.tile([1, TOK], F16)

            # ~3.5us of garbage matmuls at startup: keeps the PE busy through
            # one HAM activity window so the clock gate opens (1.2 -> 2.4
            # GHz) before the first real tile; overlaps the slab-0 DMA.
            warm = consts.tile([128, TS], F16)
            nc.vector.memset(warm[:], 0.0)
            pwarm = pwpool.tile([128, TS], F32)
            for _ in range(12):
                nc.tensor.matmul(pwarm[:], w1a[:], warm[:], start=True, stop=True)

            # Software pipeline: the z matmul for tile j-1 is issued before
            # the W1 matmuls of tile j, so the in-order PE queue never stalls
            # waiting on the scalar-engine relu of the current tile.
            ZCHUNK = 2 * SLAB  # drain zbig every 2 slabs (32 KiB per DMA)
            prev = None
            for si in range(TOK // SLAB):
                off = si * SLAB
                sa = sload.tile([128, SLAB], F16, tag="sa")
                sb = sload.tile([128, SLAB], F16, tag="sb")
                # split input loads across both HWDGE rings (sync + scalar)
                nc.sync.dma_start(sa[:], sT[0:128, off : off + SLAB])
                nc.scalar.dma_start(sb[:], sT[128:256, off : off + SLAB])
                for c in range(SLAB // TS):
                    hoff = c * TS
                    toff = off + hoff
                    if prev is not None:
                        hp, pzp, top = prev
                        nc.tensor.matmul(pzp[0:1, :], w2s[:], hp[:],
                                         start=True, stop=True)
                        nc.vector.tensor_scalar_add(
                            zbig[0:1, top : top + TS], pzp[0:1, :], 0.0)
                        if (top + TS) % ZCHUNK == 0:
                            zoff = (top + TS) - ZCHUNK
                            nc.gpsimd.dma_start(
                                zout[0:1, zoff : zoff + ZCHUNK],
                                zbig[0:1, zoff : zoff + ZCHUNK])
                    ph = phpool.tile([128, TS], F32)
                    nc.tensor.matmul(ph[:], w1a[:], sa[:, hoff : hoff + TS],
                                     start=True, stop=False)
                    nc.tensor.matmul(ph[:], w1b[:], sb[:, hoff : hoff + TS],
                                     start=False, stop=True)
                    h = hpool.tile([128, TS], F16)
                    nc.scalar.activation(h[:], ph[:], AF.Relu, bias=b1s[:])
                    pz = pzpool.tile([1, TS], F32)
                    prev = (h, pz, toff)

            hp, pzp, top = prev
            nc.tensor.matmul(pzp[0:1, :], w2s[:], hp[:], start=True, stop=True)
            nc.vector.tensor_scalar_add(zbig[0:1, top : top + TS], pzp[0:1, :], 0.0)
            zoff = TOK - ZCHUNK
            nc.gpsimd.dma_start(zout[0:1, zoff : zoff + ZCHUNK],
                              zbig[0:1, zoff : zoff + ZCHUNK])

    nc.compile()
    return nc


def _get_nc():
    global _NC
    if _NC is None:
        _NC = _build_nc()
    return _NC


def _make_in_maps(s, W1, b1, W2, b2, rnoise):
    s16 = np.asarray(s, dtype=np.float16)
    # [NCORES, D, TOK] with the contraction dim outer -> coalesced loads
    sT = np.ascontiguousarray(s16.reshape(NCORES, TOK, D).transpose(0, 2, 1))
    w1h = np.ascontiguousarray(W1, dtype=np.float16)
    b1c = np.ascontiguousarray(b1, dtype=np.float32).reshape(HID, 1)
    w2h = np.ascontiguousarray(W2[:, 1] - W2[:, 0], dtype=np.float16).reshape(HID, 1)
    return [
        {"sT": sT[c], "w1": w1h, "b1": b1c, "w2": w2h}
        for c in range(NCORES)
    ]


def run(s, W1, b1, W2, b2, rnoise, trace=False):
    nc = _get_nc()
    in_maps = _make_in_maps(s, W1, b1, W2, b2, rnoise)
    res = run_bass_kernel_spmd(nc, in_maps, list(range(NCORES)), trace=trace)
    b2d = np.float32(b2[1] - b2[0])
    z = np.concatenate(
        [r["zout"].reshape(BPC, N) for r in res.results], axis=0
    ) + b2d

    dec = z > 0
    prob = 1.0 / (1.0 + np.exp(-z.astype(np.float64)))

    # Exact recompute of borderline tokens (fp16 z error < 1.6e-3 << TAU).
    bi, ni = np.nonzero(np.abs(z) < TAU)
    if bi.size:
        sv = np.asarray(s, dtype=np.float64)[bi, ni]
        hv = np.maximum(sv @ np.asarray(W1, np.float64) + np.asarray(b1, np.float64), 0)
        zv = hv @ np.asarray(W2[:, 1] - W2[:, 0], np.float64) + float(b2d)
        dec[bi, ni] = zv > 0
        prob[bi, ni] = 1.0 / (1.0 + np.exp(-zv))

    dec = dec.astype(np.float32)
    # LB=1 row correction: a row with no active slot activates argmax(rnoise)
    rn = np.asarray(rnoise)
    for b in np.nonzero(dec.sum(axis=1) == 0)[0]:
        dec[b, np.argmax(rn[b])] = 1.0

    return (dec, prob.astype(np.float32)), res


def kernel(s, W1, b1, W2, b2, rnoise):
    (dec, prob), _ = run(s, W1, b1, W2, b2, rnoise)
    return dec, prob
